# revision 1
# baseline (speedup 1.0000x reference)
"""Trainium2 Bass kernel for CSNet14: 14-layer tiny MLP over 2M x 12 batch.

Strategy (pure data parallel, 8 cores):
  - Shard x along batch: 250000 samples/core (padded to 250880 = 49*5120).
  - On-chip layout: feature-major activations [120 partitions, 512 free],
    10 sample-groups at 12-row stride; batch streams along the free dim.
  - Each linear layer = one PE matmul with a host-built block-diagonal
    lhsT [120,120] (10 copies of W_l^T), float32r operands for 1 cyc/row.
  - Input transpose ([128 samples,12 feats] -> [12,128]) on the PE
    (transpose mode, 10 chunks per op via [128,120] tiles).
  - Epilogue relu+bias: single fused op per layer, alternating
    ScalarE activation(Relu, bias=AP) / VectorE tensor_scalar(add,max).
  - Decoder skip-adds as accumulating identity matmuls on the PE.
  - fc14 + 2-class softmax folded to d = (W14[0]-W14[1])h + (b14[0]-b14[1]);
    p0 = sigmoid(d), p1 = sigmoid(-d) written interleaved so the output
    DMA is contiguous [N,2] rows.
"""

import os
import sys
from contextlib import ExitStack

import numpy as np

for _p in ("/opt/trn_rl_repo", "/root/.axon_site/_ro/trn_rl_repo"):
    if os.path.isdir(_p) and _p not in sys.path:
        sys.path.insert(0, _p)

import concourse.bass as bass
import concourse.bacc as bacc
import concourse.mybir as mybir
import concourse.tile as tile
from concourse.bass_utils import run_bass_kernel_spmd

DIMS = [(12, 12), (12, 11), (11, 10), (10, 9), (9, 8), (8, 7), (7, 6),
        (6, 7), (7, 8), (8, 9), (9, 10), (10, 11), (11, 12), (12, 2)]
BATCH = 2_000_000
NCORES = 8
G = 10                 # sample-groups per partition block (12-row stride)
NT = 512               # free-dim samples per group per round (one PSUM bank)
TPR = NT // 128        # transposes per round
SPR = G * NT           # samples per round = 5120
B_CORE = BATCH // NCORES
ROUNDS = -(-B_CORE // SPR)          # 49
B_PAD = ROUNDS * SPR                # 250880

F32 = mybir.dt.float32
F32R = mybir.dt.float32r

# encoder layers 0..5 produce ids consumed by decoder layers 7..12
DECODER = range(7, 13)

# matmul operand dtype: bf16 | f32 | f32r
KDT = os.environ.get("KERNEL_DT", "f32r")
# ablation stages: 0=dma only, 1=+transposes+copy, 2=+matmuls, 3=full
ABLATE = int(os.environ.get("KERNEL_ABLATE", "3"))
EPI_SPLIT = os.environ.get("KERNEL_EPISPLIT", "0") == "1"
MM_DT = {"bf16": mybir.dt.bfloat16, "f32": F32, "f32r": F32R}[KDT]
# the x path (DMA, transpose, pst) stays fp32; f32r mode keeps legacy f32r x
X_DT = F32R if KDT == "f32r" else F32


def _ps_cast(ap):
    """View a PSUM AP in the transpose dtype (out must match lhsT)."""
    return ap.bitcast(X_DT) if KDT == "f32r" else ap


def build_nc(rounds=ROUNDS, b_pad=B_PAD):
    """Build and compile the single-core Bass program (run SPMD on 8 cores)."""
    nc = bacc.Bacc("TRN2", target_bir_lowering=False, debug=False)

    x_t = nc.dram_tensor("x", [b_pad, 12], X_DT, kind="ExternalInput")
    y_t = nc.dram_tensor("y", [b_pad, 2], F32, kind="ExternalOutput")
    wblk_t = nc.dram_tensor("wblk", [13, 120, 120], MM_DT, kind="ExternalInput")
    wid_t = nc.dram_tensor("wid", [120, 120], MM_DT, kind="ExternalInput")
    wd_t = nc.dram_tensor("wd", [12, 120, 128], MM_DT, kind="ExternalInput")
    bias_t = nc.dram_tensor("bias", [16, 128], F32, kind="ExternalInput")
    ident_t = nc.dram_tensor("ident", [128, 128], X_DT, kind="ExternalInput")

    x = x_t.ap()
    y = y_t.ap()

    with tile.TileContext(nc) as tc, ExitStack() as ctx:
        const = ctx.enter_context(tc.tile_pool(name="const", bufs=1))
        xin = ctx.enter_context(tc.tile_pool(name="xin", bufs=int(os.environ.get("KERNEL_XBUFS", "5"))))
        hp = ctx.enter_context(tc.tile_pool(name="h", bufs=int(os.environ.get("KERNEL_HBUFS", "6"))))
        hp2 = ctx.enter_context(tc.tile_pool(name="h2", bufs=int(os.environ.get("KERNEL_H2BUFS", "5"))))
        yp = ctx.enter_context(tc.tile_pool(name="y", bufs=4))
        psl_pool = ctx.enter_context(
            tc.tile_pool(name="psl", bufs=int(os.environ.get("KERNEL_PSL", "7")), space=bass.MemorySpace.PSUM))
        psd_pool = ctx.enter_context(
            tc.tile_pool(name="psd", bufs=int(os.environ.get("KERNEL_PSD", "1")), space=bass.MemorySpace.PSUM))

        # ---- constants (loaded once) ----
        w_sb = const.tile([120, 13 * 120], MM_DT, tag="wsb")
        nc.sync.dma_start(w_sb[:].rearrange("k (l m) -> k l m", l=13, m=120),
                          wblk_t.ap().rearrange("l k m -> k l m"))
        wid_sb = const.tile([120, 120], MM_DT, tag="wid")
        nc.sync.dma_start(wid_sb[:], wid_t.ap())
        wd_sb = const.tile([120, 12 * 128], MM_DT, tag="wd")
        nc.sync.dma_start(
            wd_sb[:].rearrange("k (q m) -> k q m", q=12, m=128),
            wd_t.ap().rearrange("q k m -> k q m"))
        bias_sb = const.tile([128, 16], F32, tag="bias")
        nc.sync.dma_start(bias_sb[:], bias_t.ap().rearrange("l k -> k l"))
        id_sb = const.tile([128, 128], X_DT, tag="ident")
        nc.sync.dma_start(id_sb[:], ident_t.ap())

        RB = int(os.environ.get("KERNEL_RB", "5"))
        REPEAT = int(os.environ.get("KERNEL_REPEAT", "1"))
        sup = None
        for rep in range(REPEAT):
          for rb in range(0, rounds, RB):
              rblock = list(range(rb, min(rb + RB, rounds)))
              # rounds are processed in PAIRS sharing [120, 2*NT] tiles so
              # each epilogue op covers two PSUM banks (fewer, larger ops)
              PW = int(os.environ.get("KERNEL_PW", "1"))
              pairs = [rblock[i:i + PW] for i in range(0, len(rblock), PW)]
              hs, idm = {}, {r: {} for r in rblock}
              half = {}
              for pr in pairs:
                  pk = pr[0]
                  # ---- load x and transpose each round of the pair ----
                  pst = psl_pool.tile([120, PW * NT], F32, tag="psl",
                                      name=f"pst_{rep}_{pk}")
                  for j, r in enumerate(pr):
                      xt = xin.tile([128, TPR * G * 12], X_DT, tag="xt",
                                    name=f"xt_{rep}_{r}")
                      nc.sync.dma_start(
                          xt[:].rearrange("p (t g i) -> p t g i", t=TPR, g=G),
                          x[r * SPR:(r + 1) * SPR, :].rearrange(
                              "(t g p) i -> p t g i", t=TPR, g=G, p=128))
                      for t in range(TPR if ABLATE >= 1 else 0):
                          nc.tensor.transpose(
                              _ps_cast(pst[:, j * NT + t * 128:
                                            j * NT + (t + 1) * 128]),
                              xt[:, t * 120:(t + 1) * 120],
                              id_sb[:])
                      half[r] = (pk, j)
                  h = hp.tile([120, PW * NT], MM_DT, tag="h0", name=f"h0_{rep}_{pk}")
                  if ABLATE >= 1:
                      if pk % 2 == 0:
                          nc.vector.tensor_copy(h[:], pst[:])
                      else:
                          nc.scalar.copy(h[:], pst[:])
                  hs[pk] = h

              for l in range(13 if ABLATE >= 2 else 0):
                  psls = {}
                  for pr in pairs:
                      pk = pr[0]
                      psl = psl_pool.tile([120, PW * NT], F32, tag="psl",
                                          name=f"psl_{rep}_{l}_{pk}")
                      for j, r in enumerate(pr):
                          nc.tensor.matmul(
                              psl[:, j * NT:(j + 1) * NT],
                              w_sb[:, l * 120:(l + 1) * 120],
                              hs[pk][:, j * NT:(j + 1) * NT],
                              start=True, stop=(l not in DECODER))
                          if l in DECODER:
                              idh = idm[r].pop(13 - l - 1)
                              nc.tensor.matmul(
                                  psl[:, j * NT:(j + 1) * NT], wid_sb[:],
                                  idh[:, j * NT:(j + 1) * NT],
                                  start=False, stop=True)
                      psls[pk] = psl
                  for pr in pairs:
                      pk = pr[0]
                      pool_l = hp if l < 6 else hp2
                      h2 = pool_l.tile([120, PW * NT], MM_DT, tag=f"h{l + 1}",
                                       name=f"h{l + 1}_{rep}_{pk}")
                      if ABLATE < 3:
                          hs[pk] = hs[pk]
                          idm[pr[0]][l] = hs[pk]
                          if PW > 1 and len(pr) > 1:
                              idm[pr[1]][l] = hs[pk]
                          continue
                      b_ap = bias_sb[0:120, l:l + 1]
                      if EPI_SPLIT:
                          cut = (PW * NT * 5) // 9
                          nc.scalar.activation(
                              h2[:, 0:cut], psls[pk][:, 0:cut],
                              mybir.ActivationFunctionType.Relu, bias=b_ap)
                          nc.vector.tensor_scalar(
                              h2[:, cut:], psls[pk][:, cut:], b_ap, 0.0,
                              mybir.AluOpType.add, mybir.AluOpType.max)
                      elif (pk + l) % 2 == 0:
                          nc.scalar.activation(
                              h2[:], psls[pk][:],
                              mybir.ActivationFunctionType.Relu, bias=b_ap)
                      else:
                          nc.vector.tensor_scalar(
                              h2[:], psls[pk][:], b_ap, 0.0,
                              mybir.AluOpType.add, mybir.AluOpType.max)
                      for r in pr:
                          if l < 6:
                              idm[r][l] = h2
                      hs[pk] = h2

              for r in rblock:
                  # ---- fc14 folded to d = wd . h13, accumulated into a
                  # shared [128, NT] psum bank over a 12-round superround:
                  # round r -> rows 32c + 10s + g (c = (r%12)//3, s = r%3)
                  c, s = (r % 12) // 3, r % 3
                  if r % 12 == 0 or sup is None:
                      psd_tile = psd_pool.tile([128, NT], F32, tag="psd",
                                               name=f"psd_{rep}_{r}")
                      sup = [psd_tile, []]
                  psd = sup[0]
                  pk, j = half[r]
                  rr = r % 12
                  if ABLATE >= 2:
                    nc.tensor.matmul(psd[:],
                                   wd_sb[:, rr * 128:(rr + 1) * 128],
                                   hs[pk][:, j * NT:(j + 1) * NT],
                                   start=(rr == 0),
                                   stop=(rr == 11 or r == rounds - 1))
                  sup[1].append(r)

                  if r % 12 == 11 or r == rounds - 1:
                      nrow = 32 * ((len(sup[1]) + 2) // 3)
                      yt = yp.tile([128, 2 * NT], F32, tag="yt")
                      if ABLATE >= 2:
                          nc.scalar.activation(
                              yt[0:nrow, 0::2], psd[0:nrow, :],
                              mybir.ActivationFunctionType.Sigmoid,
                              bias=bias_sb[0:nrow, 13:14], scale=1.0)
                          nc.scalar.activation(
                              yt[0:nrow, 1::2], psd[0:nrow, :],
                              mybir.ActivationFunctionType.Sigmoid,
                              bias=bias_sb[0:nrow, 14:15], scale=-1.0)
                      for rr in (sup[1] if ABLATE >= 2 else []):
                          cc, ss = (rr % 12) // 3, rr % 3
                          row = 32 * cc + 10 * ss
                          nc.sync.dma_start(
                              y[rr * SPR:(rr + 1) * SPR, :].rearrange(
                                  "(t g p) c -> g t p c", t=TPR, g=G, p=128),
                              yt[row:row + G, :].rearrange(
                                  "g (t p c) -> g t p c", t=TPR, p=128))
                      sup = None

    nc.compile()
    return nc


def host_prep(inputs):
    """Build the block-diagonal weight/bias blobs from the raw params."""
    Ws = [np.asarray(inputs[f"w{i + 1}"], np.float32) for i in range(14)]
    Bs = [np.asarray(inputs[f"b{i + 1}"], np.float32) for i in range(14)]

    wblk = np.zeros((13, 120, 120), np.float32)
    biasarr = np.zeros((16, 128), np.float32)
    for l in range(13):
        din, dout = DIMS[l]
        for g in range(G):
            wblk[l, 12 * g:12 * g + din, 12 * g:12 * g + dout] = Ws[l].T
            biasarr[l, 12 * g:12 * g + dout] = Bs[l]
    wid = np.eye(120, dtype=np.float32)
    wd = np.zeros((12, 120, 128), np.float32)
    wdvec = Ws[13][0] - Ws[13][1]          # [12]
    bd = float(Bs[13][0] - Bs[13][1])
    for rr in range(12):
        row = 32 * (rr // 3) + 10 * (rr % 3)
        for g in range(G):
            wd[rr, 12 * g:12 * g + 12, row + g] = wdvec
    biasarr[13, :] = bd
    biasarr[14, :] = -bd
    ident = np.eye(128, dtype=np.float32)
    if KDT == "bf16":
        import ml_dtypes
        bf = ml_dtypes.bfloat16
        wblk, wid, wd = (a.astype(bf) for a in (wblk, wid, wd))
    return dict(wblk=wblk, wid=wid, wd=wd, bias=biasarr, ident=ident)


_NC_CACHE = {}


def _get_nc():
    key = (ROUNDS, KDT, ABLATE, os.environ.get("KERNEL_REPEAT", "1"))
    if key not in _NC_CACHE:
        _NC_CACHE[key] = build_nc()
    return _NC_CACHE[key]


def kernel(**inputs):
    x = np.ascontiguousarray(np.asarray(inputs["x"], np.float32))
    consts = host_prep(inputs)

    xpad = np.zeros((NCORES, B_PAD, 12), np.float32)
    xpad[:, :B_CORE] = x.reshape(NCORES, B_CORE, 12)

    in_maps = [dict(x=xpad[c], **consts) for c in range(NCORES)]
    nc = _get_nc()
    res = run_bass_kernel_spmd(
        nc, in_maps, core_ids=list(range(NCORES)),
        trace=os.environ.get("KERNEL_TRACE", "0") == "1")
    kernel.last_results = res
    y = np.concatenate([res.results[c]["y"][:B_CORE] for c in range(NCORES)],
                       axis=0)
    return y


if __name__ == "__main__":
    nc = build_nc()
    print("compiled OK")

